# revision 1
# baseline (speedup 1.0000x reference)
"""Trainium2 Bass kernel for nn_Conv1d_NN (retrieval_knn).

Per batch: pairwise L2 distances over N=4096 positions (C=256 dims),
top-3 nearest indices per row (self + 2 NN), gather neighbor columns,
K=3 stride-3 Conv1d == sum_k W_k @ x[:, idx[:, k]] + b.

Sharding: data-parallel over batch B=16 across 8 cores (2 batches/core).

Design (v3):
- Distances via fp16 hi/lo split (hh + hl + lh products, fp32 PSUM
  accumulate; end-to-end rel err 3.4e-4 - dropping any product gives
  ~3e-2, over the 2e-2 gate).
- The -sq_j/2 bias is folded into the matmul as an augmented 2-row
  contraction (ones2 x [sqh16; sql16]), so PSUM holds
  nd = x_i.x_j - sq_j/2 directly: no eviction, no broadcast, no
  subtract pass.
- DVE max8 + max_index scan PSUM directly per [128, 2048] chunk,
  double-buffered against PE fills. Per row-block, the 2x8 chunk
  candidates merge via a tiny max8; global indices are recovered with
  eq*idx accumulation (scalar_tensor_tensor), and the conv row gathers
  are issued immediately - merge + gathers ride inside phase A's
  DVE/GpSimd slack.
- Conv: Y1^T/Y2^T = (x^T W_k^T) fp16 -> DRAM -> indirect row gather;
  Y0 computed directly in [o, n]; gathered [n, o] tiles PE-transpose
  (fp16 identity matmul) into Y0's PSUM; one ACT eviction adds bias.
- Batch emission interleaved so prep/YkT of batch b+1 and conv of
  batch b fill the PE stream between the two phase-A sweeps
  (psq + pyk + pco PSUM pools = 2+4+2 banks, exactly fitting beside
  nothing; phase A itself uses all 8).
"""

import contextlib
import sys

sys.path.insert(0, "/opt/trn_rl_repo")

import numpy as np

import concourse.bacc as bacc
import concourse.mybir as mybir
from concourse.bass import IndirectOffsetOnAxis
from concourse.bass_utils import run_bass_kernel_spmd
from concourse.masks import make_identity
from concourse.tile import TileContext

F32 = mybir.dt.float32
F16 = mybir.dt.float16
U32 = mybir.dt.uint32
AF = mybir.ActivationFunctionType
ALU = mybir.AluOpType

B, C, N, K = 16, 256, 4096, 3
NCORES = 8
BPC = B // NCORES  # batches per core
P = 128
NB = N // P        # 32 row-blocks of 128
CH = C // P        # 2 contraction halves
JT = 512           # matmul moving tile (one PSUM bank of fp32)
CHW = 1024         # phase-A scan chunk width (2 PSUM banks)
NCHK = N // CHW    # 4 chunks per row sweep
CAND = NCHK * 8    # merged candidates per row-block
W12_OFF = CH * C   # wt_sb: [W0_h0 | W0_h1 | W1_h0 W2_h0 | W1_h1 W2_h1]


class BatchCtx:
    pass


def _prep_data(nc, tc, bc):
    """Load x, build fp16 hi/lo + fp32 squares (no PE, no PSUM)."""
    b = bc.b
    for h in range(CH):
        xf = bc.scratch.tile([P, N], F32, tag=f"xf{h}", name=f"xf_{b}_{h}")
        nc.sync.dma_start(out=xf, in_=bc.x_in[b, h * P:(h + 1) * P, :])
        nc.scalar.activation(bc.xh16[h], xf, func=AF.Copy)
        nc.gpsimd.tensor_tensor(out=bc.xl16[h], in0=xf, in1=bc.xh16[h],
                                op=ALU.subtract)
        bc.xx.append(bc.scratch.tile([P, N], F32, tag=f"xx{h}",
                                     name=f"xx_{b}_{h}"))
        nc.vector.tensor_tensor(out=bc.xx[h], in0=xf, in1=xf, op=ALU.mult)


def _prep_sq(nc, tc, bc):
    """sq ones-matmul + -sq/2 fp16 pair rows."""
    b = bc.b
    with tc.tile_pool(name=f"sqc{b}", bufs=2) as sqc:
        for hj in range(N // JT):
            js = slice(hj * JT, (hj + 1) * JT)
            pq = bc.pmisc.tile([1, JT], F32, tag="pm", name=f"pq_{b}_{hj}")
            for h in range(CH):
                nc.tensor.matmul(pq[0:1, :], bc.ones_col, bc.xx[h][:, js],
                                 start=(h == 0), stop=(h == CH - 1))
            sqf = sqc.tile([1, JT], F32, tag="sqf", name=f"sqf_{b}_{hj}")
            sql = sqc.tile([1, JT], F16, tag="sql", name=f"sql_{b}_{hj}")
            nc.scalar.activation(sqf[0:1, :], pq[0:1, :], func=AF.Copy,
                                 scale=-0.5)
            nc.scalar.activation(bc.nsq[0:1, js], sqf[0:1, :], func=AF.Copy)
            nc.gpsimd.tensor_tensor(out=sql[0:1, :], in0=sqf[0:1, :],
                                    in1=bc.nsq[0:1, js], op=ALU.subtract)
            nc.sync.dma_start(out=bc.nsq[1:2, js], in_=sql[0:1, :])


def _ykt(nc, tc, bc):
    """[Y1^T | Y2^T] = x^T [W1^T | W2^T] in fp16 -> DRAM for row gathers."""
    b = bc.b
    with tc.tile_pool(name=f"yk{b}", bufs=3) as ykp:
        for ib in range(NB):
            ibs = slice(ib * P, (ib + 1) * P)
            pk = bc.pmisc.tile([P, 2 * C], F32, tag="pm", name=f"yk_{b}_{ib}")
            for h in range(CH):
                ws = slice(W12_OFF + h * 2 * C, W12_OFF + (h + 1) * 2 * C)
                nc.tensor.matmul(pk, bc.xh16[h][:, ibs], bc.wt_sb[:, ws],
                                 start=(h == 0), stop=(h == CH - 1))
            ysb = ykp.tile([P, 2 * C], F16, tag="ysb", name=f"ysb_{b}_{ib}")
            nc.vector.tensor_copy(ysb, pk)
            for ki in range(2):
                nc.sync.dma_start(out=bc.ykt_d[ki][ibs, :],
                                  in_=ysb[:, ki * C:(ki + 1) * C])


def _phase_a(nc, tc, bc):
    """Distance scan + per-block top-3 merge + conv row gathers."""
    b = bc.b
    mg = bc.mg
    with tc.tile_pool(name=f"psA{b}", bufs=2, space="PSUM") as psA:
        for ib in range(NB):
            ibs = slice(ib * P, (ib + 1) * P)
            vals = mg.tile([P, CAND], F32, tag="vals", name=f"vals_{b}_{ib}")
            idxu = mg.tile([P, CAND], U32, tag="idxu", name=f"idxu_{b}_{ib}")
            for c in range(NCHK):
                ps = psA.tile([P, CHW], F32, tag="nd", name=f"nd_{b}_{ib}_{c}")
                base = c * CHW
                groups = []
                for h in range(CH):
                    groups.append((bc.xh16[h][:, ibs],
                                   [bc.xh16[h], bc.xl16[h]]))
                    groups.append((bc.xl16[h][:, ibs], [bc.xh16[h]]))
                for gi, (stat, movs) in enumerate(groups):
                    for mi, mv in enumerate(movs):
                        for jt in range(CHW // JT):
                            js = slice(jt * JT, (jt + 1) * JT)
                            gjs = slice(base + jt * JT,
                                        base + (jt + 1) * JT)
                            nc.tensor.matmul(ps[:, js], stat, mv[:, gjs],
                                             start=(gi == 0 and mi == 0),
                                             stop=False)
                for jt in range(CHW // JT):
                    js = slice(jt * JT, (jt + 1) * JT)
                    gjs = slice(base + jt * JT, base + (jt + 1) * JT)
                    nc.tensor.matmul(ps[:, js], bc.ones2, bc.nsq[:, gjs],
                                     start=False, stop=True)
                s = slice(c * 8, c * 8 + 8)
                nc.vector.max(out=vals[:, s], in_=ps)
                nc.vector.max_index(out=idxu[:, s], in_max=vals[:, s],
                                    in_values=ps)
            # merge the NCHK x 8 candidates for this row-block
            idxg = mg.tile([P, CAND], U32, tag="idxg", name=f"ig_{b}_{ib}")
            idxf = mg.tile([P, CAND], F32, tag="idxf", name=f"if_{b}_{ib}")
            m8 = mg.tile([P, 8], F32, tag="m8", name=f"m8_{b}_{ib}")
            eqt = mg.tile([P, CAND], F32, tag="eqt", name=f"eq_{b}_{ib}")
            i12f = mg.tile([P, 2], F32, tag="i12f", name=f"i12f_{b}_{ib}")
            nc.gpsimd.tensor_tensor(out=idxg, in0=idxu, in1=bc.offt,
                                    op=ALU.add)
            nc.gpsimd.tensor_copy(idxf, idxg)
            nc.vector.max(out=m8, in_=vals)
            for col, rank in ((0, 1), (1, 2)):
                nc.vector.scalar_tensor_tensor(
                    out=eqt, in0=vals, scalar=m8[:, rank:rank + 1],
                    in1=idxf, op0=ALU.is_equal, op1=ALU.mult,
                    accum_out=i12f[:, col:col + 1])
            nc.gpsimd.tensor_copy(bc.idx12u[:, 2 * ib:2 * ib + 2], i12f)
            # conv row gathers + g1+g2 sum (GpSimd slack under phase A)
            g1 = mg.tile([P, C], F16, tag="g1", name=f"g1_{b}_{ib}")
            g2 = mg.tile([P, C], F16, tag="g2", name=f"g2_{b}_{ib}")
            nc.gpsimd.indirect_dma_start(
                out=g1, out_offset=None, in_=bc.ykt_d[0][:, :],
                in_offset=IndirectOffsetOnAxis(
                    ap=bc.idx12u[:, 2 * ib:2 * ib + 1], axis=0))
            nc.gpsimd.indirect_dma_start(
                out=g2, out_offset=None, in_=bc.ykt_d[1][:, :],
                in_offset=IndirectOffsetOnAxis(
                    ap=bc.idx12u[:, 2 * ib + 1:2 * ib + 2], axis=0))
            nc.gpsimd.tensor_tensor(out=bc.g12[:, ib * C:(ib + 1) * C],
                                    in0=g1, in1=g2, op=ALU.add)
            # conv output chunks interleave into the phase-A PE stream
            # (pmisc banks) with a 2-block lag so the DVE scans + merge +
            # gathers of the chunk's row-blocks are done before PE
            # reaches the transposes
            if ib >= 5 and (ib - 5) % (JT // P) == 0:
                _conv_chunk(nc, tc, bc, (ib - 5) // (JT // P))
        _conv_chunk(nc, tc, bc, NB // (JT // P) - 1)


def _conv_chunk(nc, tc, bc, ncn):
    """Y0 + transposed gather accumulate + biased eviction + out DMA
    for one 512-wide output chunk."""
    b = bc.b
    nsl = slice(ncn * JT, (ncn + 1) * JT)
    for oh in range(CH):
        ohs = slice(oh * P, (oh + 1) * P)
        pso = bc.pmisc.tile([P, JT], F32, tag="pm",
                            name=f"pso_{b}_{ncn}_{oh}")
        for h in range(CH):
            ws = slice(h * C + oh * P, h * C + (oh + 1) * P)
            nc.tensor.matmul(pso, bc.wt_sb[:, ws], bc.xh16[h][:, nsl],
                             start=(h == 0), stop=False)
        for nb4 in range(JT // P):
            ib = ncn * (JT // P) + nb4
            bs = slice(nb4 * P, (nb4 + 1) * P)
            gsl = slice(ib * C + oh * P, ib * C + (oh + 1) * P)
            nc.tensor.matmul(pso[:, bs], bc.g12[:, gsl], bc.ident16,
                             start=False, stop=True)
        osb = bc.mg.tile([P, JT], F32, tag="osb", name=f"osb_{b}_{ncn}_{oh}")
        nc.scalar.activation(osb, pso, func=AF.Identity,
                             bias=bc.biasc[oh][:, 0:1])
        nc.sync.dma_start(out=bc.out_t[b, ohs, nsl], in_=osb)


def build():
    nc = bacc.Bacc(None, target_bir_lowering=False)
    x_in = nc.dram_tensor("x", [BPC, C, N], F32, kind="ExternalInput")
    wt_in = nc.dram_tensor("wt", [K, C, C], F16, kind="ExternalInput")
    bias_in = nc.dram_tensor("bias", [C, 1], F32, kind="ExternalInput")
    out_t = nc.dram_tensor("out", [BPC, C, N], F32, kind="ExternalOutput")

    with TileContext(nc) as tc, contextlib.ExitStack() as es:
        constp = es.enter_context(tc.tile_pool(name="const", bufs=1))
        ident16 = constp.tile([P, P], F16)
        ones_col = constp.tile([P, 1], F32)
        ones2 = constp.tile([2, P], F16)
        wt_sb = constp.tile([P, K * CH * C], F16)
        offt = constp.tile([P, CAND], U32)
        biasc = [constp.tile([P, 1], F32, tag=f"bc{oh}", name=f"bc{oh}")
                 for oh in range(CH)]
        make_identity(nc, ident16)
        nc.vector.memset(ones_col, 1.0)
        nc.vector.memset(ones2, 1.0)
        nc.gpsimd.iota(offt, pattern=[[CHW, NCHK], [0, 8]], base=0,
                       channel_multiplier=0)
        for oh in range(CH):
            nc.sync.dma_start(out=biasc[oh],
                              in_=bias_in[oh * P:(oh + 1) * P, :])
        for h in range(CH):
            nc.sync.dma_start(out=wt_sb[:, h * C:(h + 1) * C],
                              in_=wt_in[0, h * P:(h + 1) * P, :])
            for k in (1, 2):
                ws = slice(W12_OFF + h * 2 * C + (k - 1) * C,
                           W12_OFF + h * 2 * C + k * C)
                nc.sync.dma_start(out=wt_sb[:, ws],
                                  in_=wt_in[k, h * P:(h + 1) * P, :])

        xb = es.enter_context(tc.tile_pool(name="xb", bufs=1))
        mg = es.enter_context(tc.tile_pool(name="mg", bufs=4))
        scratch = es.enter_context(tc.tile_pool(name="scratch", bufs=1))
        pmisc = es.enter_context(
            tc.tile_pool(name="pmisc", bufs=4, space="PSUM"))
        ydr = es.enter_context(tc.tile_pool(name="ydr", bufs=1, space="DRAM"))
        bcs = []
        for b in range(BPC):
            bc = BatchCtx()
            bc.b, bc.mg, bc.scratch, bc.xx = b, mg, scratch, []
            bc.pmisc, bc.out_t = pmisc, out_t
            bc.x_in, bc.wt_sb, bc.biasc = x_in, wt_sb, biasc
            bc.ident16, bc.ones_col, bc.ones2, bc.offt = \
                ident16, ones_col, ones2, offt
            bc.xh16 = [xb.tile([P, N], F16, tag=f"xh{h}_{b}",
                               name=f"xh_{b}_{h}") for h in range(CH)]
            bc.xl16 = [xb.tile([P, N], F16, tag=f"xl{h}_{b}",
                               name=f"xl_{b}_{h}") for h in range(CH)]
            bc.nsq = xb.tile([2, N], F16, tag=f"nsq_{b}", name=f"nsq_{b}")
            bc.idx12u = xb.tile([P, NB * 2], U32, tag=f"i12u_{b}",
                                name=f"i12u_{b}")
            bc.g12 = xb.tile([P, NB * C], F16, tag=f"g12_{b}",
                             name=f"g12_{b}")
            bc.ykt_d = [ydr.tile([N, C], F16, tag=f"y{k}t_{b}",
                                 name=f"y{k}t_{b}") for k in (1, 2)]
            bcs.append(bc)

        # Emission order == per-engine execution order. All input prep
        # and ykt for BOTH batches run up front (engines other than PE
        # absorb the chains; PE does ykt/sq warm-up work), then the PE
        # stream is phaseA(0)+conv(0), phaseA(1)+conv(1) with no idle
        # long enough to re-throttle HAM. psA (2-bank chunks x 2 bufs)
        # and the shared 4-bank pmisc pool coexist within 8 PSUM banks,
        # so no pool transition blocks the PE stream.
        for b in range(BPC):
            _prep_data(nc, tc, bcs[b])
        # dummy matmuls keep the PE HAM monitor busy (warm 2.4 GHz)
        # while the input DMA/cast chains run
        warm = pmisc.tile([P, P], F32, tag="pm", name="warm")
        for _ in range(96):
            nc.tensor.matmul(warm, ident16, ident16, start=True, stop=True)
        for b in range(BPC):
            _ykt(nc, tc, bcs[b])
            _prep_sq(nc, tc, bcs[b])
        for b in range(BPC):
            _phase_a(nc, tc, bcs[b])
    nc.compile()
    return nc


_NC = None


def _get_nc():
    global _NC
    if _NC is None:
        _NC = build()
    return _NC


def make_in_maps(x, W, b):
    x = np.ascontiguousarray(x, dtype=np.float32)
    wt = np.ascontiguousarray(np.transpose(W, (2, 1, 0))).astype(np.float16)
    bias = np.ascontiguousarray(b, dtype=np.float32).reshape(C, 1)
    return [
        {"x": np.ascontiguousarray(x[i * BPC:(i + 1) * BPC]),
         "wt": wt, "bias": bias}
        for i in range(NCORES)
    ]


def kernel(x, W, b):
    nc = _get_nc()
    in_maps = make_in_maps(x, W, b)
    res = run_bass_kernel_spmd(nc, in_maps, core_ids=list(range(NCORES))).results
    return np.concatenate([r["out"] for r in res], axis=0)

